# revision 8
# baseline (speedup 1.0000x reference)
"""GCN layer kernel for Trainium2 (Bass/Tile), data-parallel over batch.

Per core (one batch element):
    out = relu(D^-1/2 A D^-1/2 (X W^T + b))

Host-side prep per core (numpy: dtype/layout marshaling + the O(N^2) deg fold):
  - ATd = (D^-1/2 A)^T cast to bf16: A transposed (the tensor engine contracts
    over partitions, so A's contraction index must live on partitions), with
    the output-row scale D^-1/2 folded in so the PSUM drain is a pure relu.
    bf16 halves HBM traffic vs the f32 A load that bottlenecked the baseline.
  - X^T, W^T, b cast to bf16; d = deg^-1/2 as a [128, 16] f32 column table for
    the y = d * (XW^T + b) scale (deg needs full A rows, which live across all
    16 device tiles; host computes it to avoid a load/matmul barrier).
  - Output returns transposed [256, 2048] bf16; host casts + transposes back.

Device schedule (per core), paced by the 16 x 1 MB A^T tile DMA stream:
  - HWDGE loads on the SP ring: d/b/X^T/W^T first, then the A^T tiles.
  - mm1 phase: y_k = d_k * (X_k W^T + b) for all 16 k, cycling 8 PSUM regions
    across 4 banks so the PE never waits on the ACT/DVE drain round-trip
    (with only 2 regions the in-order PE queue stalls on the write-after-read
    ping-pong, which stretched every later product round). Drains alternate
    ACT / DVE. Doubles as the PE HAM warmup.
  - main matmul, transposed-output form: out^T[o, r] = sum_c y[c, o] ATd[c, r]
    with y chunks stationary and ATd the moving operand in 512-wide slices:
    per tile k just 8 matmuls of N=512 into the 8 PSUM banks (o-chunk x
    r-quarter), LDWEIGHTS hidden under the 512-col streams, one clean
    accumulation group per bank. 4 of the banks are the recycled mm1 banks.
  - tail: relu drains (alternating ACT / DVE) into a bf16 staging tile, 4 x
    256 KB output DMAs on the by-then-idle SP ring.
"""

from contextlib import ExitStack

import ml_dtypes
import numpy as np

import concourse.bacc as bacc
import concourse.mybir as mybir
import concourse.tile as tile
from concourse.bass_utils import run_bass_kernel_spmd

B = 8
N = 2048
F = 256
P = 128
NT = N // P  # 16 A^T row tiles
FT = F // P  # 2 feature tiles
RQ = 4  # 512-wide r-quarters per A^T tile
RW = N // RQ  # 512
F32 = mybir.dt.float32
BF16 = mybir.dt.bfloat16
COPY = mybir.ActivationFunctionType.Copy
RELU = mybir.ActivationFunctionType.Relu
MULT = mybir.AluOpType.mult
MAX = mybir.AluOpType.max
BF = ml_dtypes.bfloat16


def _emit(ctx: ExitStack, tc: tile.TileContext, AT, XT, WT, BIAS, DCOL, OUT):
    nc = tc.nc

    const = ctx.enter_context(tc.tile_pool(name="const", bufs=1))
    atp = ctx.enter_context(tc.tile_pool(name="atp", bufs=1))
    psum = ctx.enter_context(tc.tile_pool(name="psum", bufs=4, space="PSUM"))

    xt_sb = const.tile([P, FT * N], BF16, tag="xt")
    wt_sb = const.tile([P, FT * F], BF16, tag="wt")
    dcol = const.tile([P, NT], F32, tag="dcol")
    bias_sb = const.tile([1, F], BF16, tag="bias")
    ones1 = const.tile([1, P], BF16, tag="ones")
    y_big = const.tile([P, NT * F], BF16, tag="y")
    out_t = const.tile([P, FT * N], BF16, tag="out")
    at_big = atp.tile([P, NT * N], BF16, tag="at")

    # input DMAs (HWDGE, SP ring): mm1 operands first, then A^T row tiles
    nc.sync.dma_start(out=dcol[:, :], in_=DCOL[:, :])
    nc.sync.dma_start(out=bias_sb[:, :], in_=BIAS[:, :])
    for phi in range(FT):
        nc.sync.dma_start(
            out=xt_sb[:, phi * N : (phi + 1) * N], in_=XT[phi * P : (phi + 1) * P, :]
        )
        nc.sync.dma_start(
            out=wt_sb[:, phi * F : (phi + 1) * F], in_=WT[phi * P : (phi + 1) * P, :]
        )
    for k in range(NT):
        nc.sync.dma_start(
            out=at_big[:, k * N : (k + 1) * N], in_=AT[k * P : (k + 1) * P, :]
        )

    nc.vector.memset(ones1[:, :], 1.0)

    # ---- mm1: y_k for all k through 8 regions in 4 banks (deep pipeline) ----
    mm1 = [psum.tile([P, 2 * F], F32, tag="mm1", bufs=4, name=f"mm1_{i}") for i in range(4)]
    for k in range(NT):
        h = (k // 4) % 2
        reg = mm1[k % 4][:, h * F : (h + 1) * F]
        nc.tensor.matmul(
            reg, ones1[:, :], bias_sb[:, :], start=True, stop=False,
            skip_group_check=True,
        )
        for phi in range(FT):
            nc.tensor.matmul(
                reg,
                xt_sb[:, phi * N + k * P : phi * N + (k + 1) * P],
                wt_sb[:, phi * F : (phi + 1) * F],
                start=False,
                stop=(phi == FT - 1),
                skip_group_check=True,
            )
        dst = y_big[:, k * F : (k + 1) * F]
        if k % 2 == 0:
            nc.scalar.activation(dst, reg, COPY, scale=dcol[:, k : k + 1])
        else:
            nc.vector.tensor_scalar(
                out=dst, in0=reg, scalar1=dcol[:, k : k + 1], scalar2=None, op0=MULT
            )

    # ---- main matmul, transposed output: 8 banks = (o-chunk, r-quarter) ----
    banks = {}
    for oc in range(FT):
        for rc in range(0, RQ, 2):
            banks[(oc, rc)] = psum.tile(
                [P, 2 * F], F32, tag="bank", bufs=4, name=f"bank_{oc}_{rc}"
            )
    for oc in range(FT):  # recycled mm1 banks
        for rc in range(1, RQ, 2):
            banks[(oc, rc)] = psum.tile(
                [P, 2 * F], F32, tag="mm1", bufs=4, name=f"bank_{oc}_{rc}"
            )

    for k in range(NT):
        for rc in range(RQ):
            for oc in range(FT):
                for hf in range(2):
                    nc.tensor.matmul(
                        banks[(oc, rc)][:, hf * F : (hf + 1) * F],
                        y_big[:, k * F + oc * P : k * F + (oc + 1) * P],
                        at_big[
                            :,
                            k * N + rc * RW + hf * F : k * N + rc * RW + (hf + 1) * F,
                        ],
                        start=(k == 0 and hf == 0),
                        stop=(k == NT - 1),
                        skip_group_check=True,
                    )

    # ---- drains (pure relu; d_r folded into ATd) + 4 output DMAs ----
    for oc in range(FT):
        for rc in range(RQ):
            src = banks[(oc, rc)][:, :RW]
            dst = out_t[:, oc * N + rc * RW : oc * N + (rc + 1) * RW]
            if rc % 2 == 0:
                nc.scalar.activation(dst, src, RELU)
            else:
                nc.vector.tensor_scalar(
                    out=dst, in0=src, scalar1=0.0, scalar2=None, op0=MAX
                )
            if rc % 2 == 1:
                half = rc // 2
                nc.sync.dma_start(
                    out=OUT[oc * P : (oc + 1) * P, half * 2 * RW : (half + 1) * 2 * RW],
                    in_=out_t[
                        :, oc * N + half * 2 * RW : oc * N + (half + 1) * 2 * RW
                    ],
                )


_cached_nc = None


def _build():
    nc = bacc.Bacc("TRN2", target_bir_lowering=False, debug=False)
    AT = nc.dram_tensor("at", [N, N], BF16, kind="ExternalInput").ap()
    XT = nc.dram_tensor("xt", [F, N], BF16, kind="ExternalInput").ap()
    WT = nc.dram_tensor("wt", [F, F], BF16, kind="ExternalInput").ap()
    BIAS = nc.dram_tensor("bias", [1, F], BF16, kind="ExternalInput").ap()
    DCOL = nc.dram_tensor("dcol", [P, NT], F32, kind="ExternalInput").ap()
    OUT = nc.dram_tensor("out", [F, N], BF16, kind="ExternalOutput").ap()
    with tile.TileContext(nc) as tc:
        with ExitStack() as ctx:
            _emit(ctx, tc, AT, XT, WT, BIAS, DCOL, OUT)
    nc.compile()
    return nc


def get_nc():
    global _cached_nc
    if _cached_nc is None:
        _cached_nc = _build()
    return _cached_nc


def make_in_maps(node_features, adj_matrix, W, b):
    node_features = np.asarray(node_features, dtype=np.float32)
    adj_matrix = np.asarray(adj_matrix, dtype=np.float32)
    wt = np.ascontiguousarray(np.asarray(W, dtype=np.float32).T.astype(BF))
    bias = np.ascontiguousarray(
        np.asarray(b, dtype=np.float32).astype(BF).reshape(1, F)
    )
    maps = []
    for c in range(B):
        adj = adj_matrix[c]
        deg = adj.sum(axis=1, dtype=np.float32)
        with np.errstate(divide="ignore"):
            d = deg**-0.5
        d = np.where(np.isfinite(d), d, 0.0).astype(np.float32)
        maps.append(
            {
                # (D^-1/2 A)^T: row scale folded in before the bf16 cast
                "at": np.ascontiguousarray((adj * d[:, None]).astype(BF).T),
                "xt": np.ascontiguousarray(node_features[c].T.astype(BF)),
                "wt": wt,
                "bias": bias,
                "dcol": np.ascontiguousarray(d.reshape(NT, P).T),
            }
        )
    return maps


def unpack_out(arr):
    """Device output [F, N] bf16 -> full-precision [N, F] f32."""
    return np.ascontiguousarray(np.asarray(arr).astype(np.float32).T)


def kernel(node_features, adj_matrix, W, b):
    nc = get_nc()
    in_maps = make_in_maps(node_features, adj_matrix, W, b)
    res = run_bass_kernel_spmd(nc, in_maps, core_ids=list(range(B)))
    return np.stack([unpack_out(r["out"]) for r in res.results], axis=0)
